# revision 10
# baseline (speedup 1.0000x reference)
"""Trainium2 Bass kernel for the Air3D CNF ROM model (nn_Air3DCNFROM).

Model: out[b] = lx(x_b) + tau_b * u_b where u = MLP([fourier(x), alpha(tau)])
(106 -> 512 -> 512 -> 512 -> 1, tanh), alpha = linear interp at tau of the
101-step RK4 latent trajectory (identical for every sample since alpha0 = 0).

Device, per 512-sample tile: fourier features (DVE range reduction in turns +
ACT Sin on the one table set shared with Tanh), the bf16 tanh decoder on
PE/ACT, an off-PE u-strip relayout (strip -> DRAM -> [128,4] scatter DMA,
latency hidden one group downstream; last group uses PE transposes to keep
the tail short), and the final out = lx + tau*u combine with per-group
output DMAs. Host: RK4 trajectory + its interp at tau ([10,B] bf16),
lx = |x_xy| - R, weight packing/cast, and a sample permutation such that
every DMA is a dense [rows, cols] tile (device sample 512t+128c+p lives at
out[p, 4t+c]; v1's (t c p) rearranges drained 25k 4-byte packets for ~20us).

The two scheduling insights worth keeping (measured via NTFF/HAM traces):
  * The PE clock is HAM-gated: 1.2 GHz until ~3.4us of sustained high array
    activity, re-throttling after any low-activity window. v1 spent 25-55us
    at half clock. Fixes: a warmup burst of full-array dummy matmuls under
    the input DMAs, and group-start interleaving (L1s of the new group
    braided with its first L2s) so array activity never dips. HAM now stays
    at 8/8 for the whole run.
  * Only 8 HWDGE rings exist and a 9th descriptor's issue blocks its QUEUE
    until a ring frees: bulk prefetches (w1/w2, bc96 tails) are issued on
    the idle Sync queue between the first tiles' emissions, never ahead of
    ring-critical loads and never on the busy ACT queue.

Matmuls are bf16 (PSUM f32; 1 cycle/row warm), except L2 which runs in
fp8-e4m3 DoubleRow mode (0.5 cycles/row, contraction 256 per instruction,
weights pre-interleaved on host as w1p[p, kp*1024+kk*512+n] =
w1[(2kp+kk)*128+p, n]). Host-simulated and HW-measured error agree at
rel 1.06e-2 vs the 2e-2 gate (fp8 on BOTH mid layers simulates to 1.8e-2 -
too close to the gate, so only L2 is quantized; its error is contracted by
the two downstream tanh layers).

Distribution: pure data parallel over 8 NeuronCores (batch 65536 -> 8x8192).
L1 folds its bias into the matmul (ones row 96 of h0 x bias row 96 of w0;
contraction 106->107 is free since matmul cost is column count) which makes
its tanh bias-free and pairable over 2-bank psum: 2 ACT ops per tile
instead of 4. Post-fp8-L2 the scalar engine is the pacing engine, so this
is a direct span cut.

Measured: 172.3us HW exec (v1 f32r baseline: 255us), rel err 1.06e-2.
"""
import numpy as np
import ml_dtypes

import concourse.bass as bass
import concourse.tile as tile
from concourse.tile import add_dep_helper
from concourse import bacc, mybir
import concourse.hw_specs as _hw_specs
from concourse.bass_utils import run_bass_kernel_spmd

# Route Tanh and Sin to the one ACT table set that holds BOTH, so the scalar
# engine never swaps tables (~1.3us each swap).
_orig_get_activation_tables = _hw_specs.get_activation_tables


def _patched_get_activation_tables(arch):
    t = _orig_get_activation_tables(arch)
    both = t.get("silu_and_others", set())
    AFT = mybir.ActivationFunctionType
    if AFT.Tanh in both and AFT.Sin in both:
        for name, fns in t.items():
            if name != "silu_and_others":
                fns.discard(AFT.Tanh)
                fns.discard(AFT.Sin)
    return t


_hw_specs.get_activation_tables = _patched_get_activation_tables
bacc.get_activation_tables = _patched_get_activation_tables

F32 = mybir.dt.float32
F8 = mybir.dt.float8e4
BF16 = mybir.dt.bfloat16
I32 = mybir.dt.int32
AF = mybir.ActivationFunctionType
ALU = mybir.AluOpType

N_CORES = 8
B = 65536
B_SHARD = B // N_CORES
NT = 512
LAT = 10
STEPS = 101
DTAU = np.float32(0.01)
RADIUS = 0.25
N_FREQS = 16
MAX_FREQ = 10.0
PI2 = float(2.0 * np.pi)

# misc f32 tensor column map
MF_B0 = 0            # [128, 4]
MF_B1 = 4
MF_B2 = 8
MF_F96 = 12          # [96, 1]
MF_PH96 = 13         # [96, 1]
MF_ID4 = 14          # [4, 4] f32 identity (for the [G,128] transposes)
MF_LX = 18           # [128, 64]
MF_TAU = 82          # [128, 64]
MF_COLS = 146


def _host_traj(pn_w0, pn_b0, pn_w1, pn_b1, pn_w2, pn_b2):
    """RK4 scan of the pnode ODE for a single zero-initialized latent,
    mirroring the reference's float32 arithmetic."""
    f32 = np.float32
    half_dtau = f32(0.5) * DTAU
    dtau6 = f32(0.01 / 6.0)
    two = f32(2.0)
    ts = np.linspace(0.0, 1.0, STEPS, dtype=np.float32)

    def f(t, a):
        inp = np.concatenate([a, np.full((1, 1), t, np.float32)], axis=1)
        h = np.tanh(inp @ pn_w0 + pn_b0)
        h = np.tanh(h @ pn_w1 + pn_b1)
        return h @ pn_w2 + pn_b2

    a = np.zeros((1, LAT), np.float32)
    traj = np.empty((STEPS, LAT), np.float32)
    traj[0] = a
    for i in range(STEPS - 1):
        t = ts[i]
        k1 = f(t, a)
        k2 = f(t + half_dtau, a + half_dtau * k1)
        k3 = f(t + half_dtau, a + half_dtau * k2)
        k4 = f(t + DTAU, a + DTAU * k3)
        a = a + dtau6 * (k1 + two * k2 + two * k3 + k4)
        traj[i + 1] = a
    return traj


def build_kernel(b_shard: int, b3_val: float, detect_races: bool = True,
                 biases_zero: bool = False):
    n_tiles = b_shard // NT
    G = min(4, n_tiles)
    assert n_tiles % G == 0
    q = n_tiles * 4  # out columns

    nc = bacc.Bacc("TRN2", target_bir_lowering=False, debug=False,
                   detect_race_conditions=detect_races)

    # ---- DRAM I/O
    d_bc96 = nc.dram_tensor("bc96", [96, b_shard], F32, kind="ExternalInput").ap()
    d_alph = nc.dram_tensor("alph", [LAT + 1, b_shard], BF16, kind="ExternalInput").ap()
    d_w0 = nc.dram_tensor("w0", [107, 512], BF16, kind="ExternalInput").ap()
    d_w1p = nc.dram_tensor("w1p", [128, 2048], F8, kind="ExternalInput").ap()
    d_w2p = nc.dram_tensor("w2p", [128, 2048], BF16, kind="ExternalInput").ap()
    d_mf = nc.dram_tensor("mf", [128, MF_COLS], F32, kind="ExternalInput").ap()
    d_w3 = nc.dram_tensor("w3", [128, 4], BF16, kind="ExternalInput").ap()
    d_out = nc.dram_tensor("out", [128, q], F32, kind="ExternalOutput").ap()
    d_scr = nc.dram_tensor("scr", [n_tiles, NT], F32, kind="Internal").ap()

    with tile.TileContext(nc) as tc:
        with tc.tile_pool(name="res", bufs=1) as res, \
             tc.tile_pool(name="tmp", bufs=2) as tmp, \
             tc.tile_pool(name="ps", bufs=2, space="PSUM") as ps, \
             tc.tile_pool(name="psl1", bufs=1, space="PSUM") as psl1, \
             tc.tile_pool(name="psl2", bufs=1, space="PSUM") as psl2, \
             tc.tile_pool(name="psl3", bufs=1, space="PSUM") as psl3:

            # ---- resident tensors; issue order = ramp priority.
            # Critical path for tile 0: mf (f96/ph96) + bc96[:, :512] ->
            # fourier; alph -> h0 alpha rows; w0 -> L1.
            mf_sb = res.tile([128, MF_COLS], F32, name="mf_sb")
            nc.sync.dma_start(mf_sb[:], d_mf)
            bc96_sb = res.tile([96, b_shard], F32, name="bc96_sb")
            nc.sync.dma_start(bc96_sb[:, 0:NT], d_bc96[:, 0:NT])
            alph_sb = res.tile([LAT + 1, b_shard], BF16, name="alph_sb")
            nc.sync.dma_start(alph_sb[:], d_alph)
            w3_sb = res.tile([128, 4], BF16, name="w3_sb")
            nc.sync.dma_start(w3_sb[:], d_w3)
            w0_sb = res.tile([107, 512], BF16, name="w0_sb")
            nc.scalar.dma_start(w0_sb[:], d_w0)
            w1_sb = res.tile([128, 2048], F8, name="w1_sb")
            w2_sb = res.tile([128, 2048], BF16, name="w2_sb")

            f96_v = mf_sb[0:96, MF_F96:MF_F96 + 1]
            ph96_v = mf_sb[0:96, MF_PH96:MF_PH96 + 1]
            lx_v = mf_sb[:, MF_LX:MF_LX + q]
            tau_v = mf_sb[:, MF_TAU:MF_TAU + q]

            ident4_v = mf_sb[0:4, MF_ID4:MF_ID4 + 4]
            u_sb = res.tile([128, q], F32, name="u_sb")
            fin = res.tile([128, q], F32, name="fin")

            # PE warmup: the HAM clock gate keeps the PE at 1.2 GHz until it
            # sees ~3.4us of sustained matmul activity, and re-throttles
            # after a low-activity window. The dummies run while the input
            # DMAs land and during the DVE-bound pipeline fill.
            scratch = res.tile([128, 512], BF16, name="scratch")
            nc.vector.memset(scratch[:], 0.25)

            def emit_warm(tag, n):
                for i in range(n):
                    pw = ps.tile([128, NT], F32, tag="mm", name=f"warm_{tag}_{i}")
                    nc.tensor.matmul(pw[:], scratch[:, 0:128], scratch[:],
                                     start=True, stop=True)

            h0 = [res.tile([128, NT], BF16, name=f"h0_{s}") for s in range(G)]
            h1 = [res.tile([128, 4 * NT], F8, name=f"h1_{s}") for s in range(G)]
            h2 = [res.tile([128, 4 * NT], BF16, name=f"h2_{s}") for s in range(G)]
            h3 = [res.tile([128, 4 * NT], BF16, name=f"h3_{s}") for s in range(G)]

            strips: dict = {}

            def emit_f(t):
                s = t % G
                cs = bass.ts(t, NT)
                proj = tmp.tile([96, NT], F32, tag="proj", name=f"proj_{t}")
                nc.vector.tensor_scalar(proj[:], bc96_sb[:, cs], f96_v,
                                        ph96_v, op0=ALU.mult, op1=ALU.add)
                ri = tmp.tile([96, NT], I32, tag="ri", name=f"ri_{t}")
                nc.vector.tensor_copy(ri[:], proj[:])
                rf = tmp.tile([96, NT], F32, tag="rf", name=f"rf_{t}")
                nc.vector.tensor_copy(rf[:], ri[:])
                rr = tmp.tile([96, NT], F32, tag="rr", name=f"rr_{t}")
                nc.vector.tensor_sub(rr[:], proj[:], rf[:])
                # rrf = (rr > 0.5) - rr: folds to [-0.5, 0.5] with a sign flip
                # compensated by negating w0's fourier rows on the host.
                rrf = tmp.tile([96, NT], F32, tag="rrf", name=f"rrf_{t}")
                nc.vector.scalar_tensor_tensor(rrf[:], rr[:], 0.5, rr[:],
                                               op0=ALU.is_gt, op1=ALU.subtract)
                nc.scalar.activation(h0[s][0:96, :], rrf[:], AF.Sin, scale=PI2)

            def emit_h(t):
                s = t % G
                nc.vector.tensor_copy(h0[s][96:97 + LAT, :],
                                      alph_sb[:, bass.ts(t, NT)])

            def emit_l1(t):
                s = t % G
                # bias folded into the matmul (ones row 96 of h0 x b0 row 96
                # of w0) -> bias-free tanh pairs over 2-bank psum, halving
                # the ACT op count for L1 (ACT is the wall post-fp8-L2).
                for half in range(2):
                    p = psl1.tile([128, 2 * NT], F32, tag="l1",
                                  name=f"p_l1_{t}_{half}")
                    for mm_ in range(2):
                        m = 2 * half + mm_
                        nc.tensor.matmul(p[:, bass.ts(mm_, NT)],
                                         w0_sb[:, bass.ts(m, 128)],
                                         h0[s][0:107, :],
                                         start=True, stop=True)
                    nc.scalar.activation(
                        h1[s][:, half * 2 * NT:(half + 1) * 2 * NT], p[:],
                        AF.Tanh)

            def drain_pair(p, hout_s, half, bcol):
                # one bias-free [128,1024] tanh per pair when the model's
                # biases are all zero (true for this problem); otherwise two
                # biased [128,512] tanhs over the same 2-bank psum tile.
                if biases_zero:
                    nc.scalar.activation(
                        hout_s[:, half * 2 * NT:(half + 1) * 2 * NT], p[:],
                        AF.Tanh)
                else:
                    for mm_ in range(2):
                        m = 2 * half + mm_
                        nc.scalar.activation(
                            hout_s[:, bass.ts(m, NT)],
                            p[:, bass.ts(mm_, NT)], AF.Tanh,
                            bias=mf_sb[:, bcol + m:bcol + m + 1])

            def emit_l23(t, layer):
                s = t % G
                if layer == 2:
                    # fp8 DoubleRow: contraction 256 per instruction
                    # (2 k-tiles packed along the free dims), 0.5 cycles/row.
                    h1v = h1[s][:].rearrange("p (kp kk n) -> p kp kk n",
                                             kp=2, kk=2)
                    w1v = w1_sb[:].rearrange("p (kp kk m) -> p kp kk m",
                                             kp=2, kk=2)
                    for half in range(2):
                        p = psl2.tile([128, 2 * NT], F32, tag="l2",
                                      name=f"p_l2_{t}_{half}")
                        for mm_ in range(2):
                            m = 2 * half + mm_
                            for kp in range(2):
                                nc.tensor.matmul(
                                    p[:, bass.ts(mm_, NT)],
                                    w1v[:, kp, :, bass.ts(m, 128)],
                                    h1v[:, kp, :, :],
                                    start=(kp == 0), stop=(kp == 1),
                                    perf_mode=mybir.MatmulPerfMode.DoubleRow)
                        drain_pair(p, h2[s], half, MF_B1)
                    return
                for half in range(2):
                    p = psl3.tile([128, 2 * NT], F32, tag="l3",
                                  name=f"p_l3_{t}_{half}")
                    for mm_ in range(2):
                        m = 2 * half + mm_
                        for k in range(4):
                            nc.tensor.matmul(
                                p[:, bass.ts(mm_, NT)],
                                w2_sb[:, k * NT + m * 128:k * NT + (m + 1) * 128],
                                h2[s][:, bass.ts(k, NT)],
                                start=(k == 0), stop=(k == 3))
                    drain_pair(p, h3[s], half, MF_B2)

            strip_dmas: dict = {}

            def emit_l4_mm(t, last_group=False):
                s = t % G
                p_u = ps.tile([128, NT], F32, tag="mm", name=f"p_u_{t}")
                for k in range(4):
                    nc.tensor.matmul(p_u[0:1, :], w3_sb[:, k:k + 1],
                                     h3[s][:, bass.ts(k, NT)],
                                     start=(k == 0), stop=(k == 3))
                strip = tmp.tile([1, NT], F32, tag="strip", name=f"strip_{t}",
                                 bufs=5)
                nc.vector.tensor_scalar(strip[:], p_u[0:1, :], float(b3_val),
                                        None, op0=ALU.add)
                strips[t] = strip
                if not last_group:
                    # park the strip in DRAM; the group gather re-reads it in
                    # [128, 4] orientation one group later (latency hidden),
                    # keeping the relayout off the busy PE.
                    strip_dmas[t] = nc.sync.dma_start(d_scr[t:t + 1, :],
                                                      strip[:])

            def emit_gather(g, last_group=False):
                if last_group:
                    # tail-latency-critical: PE transposes, no DRAM round trip
                    for t in range(g * G, (g + 1) * G):
                        strip = strips.pop(t)
                        p_t = ps.tile([128, 512], F32, tag="mm",
                                      name=f"p_t_{t}")
                        for c in range(4):
                            nc.tensor.transpose(p_t[:, c:c + 1],
                                                strip[0:1, bass.ts(c, 128)],
                                                ident4_v[0:1, 0:1])
                        nc.vector.tensor_copy(u_sb[:, bass.ts(t, 4)],
                                              p_t[:, 0:4])
                    return
                gd = nc.sync.dma_start(
                    u_sb[:, bass.ts(g, 4 * G)],
                    d_scr[bass.ts(g, G), :].rearrange(
                        "t (c p) -> p (t c)", p=128))
                for t in range(g * G, (g + 1) * G):
                    strips.pop(t)
                    add_dep_helper(gd.ins, strip_dmas.pop(t).ins,
                                   reason="dram-bounce-RAW")

            def emit_epilogue(g):
                cols = bass.ts(g, 4 * G)
                mu = tmp.tile([128, 4 * G], F32, tag="mu", name=f"mu_{g}")
                nc.vector.tensor_tensor(mu[:], tau_v[:, cols], u_sb[:, cols],
                                        op=ALU.mult)
                nc.vector.tensor_tensor(fin[:, cols], mu[:], lx_v[:, cols],
                                        op=ALU.add)
                nc.sync.dma_start(d_out[:, cols], fin[:, cols])

            n_groups = n_tiles // G
            emit_warm("a", 30)
            # alpha copies first: they gate L1 and need only the alph DMA;
            # left inline the scheduler parks them behind later fourier ops.
            for t in range(G):
                emit_h(t)
            emit_f(0)
            emit_l1(0)
            nc.sync.dma_start(bc96_sb[:, NT:2 * NT], d_bc96[:, NT:2 * NT])
            emit_warm("f0", 6)
            emit_f(1)
            nc.sync.dma_start(bc96_sb[:, 2 * NT:4 * NT],
                              d_bc96[:, 2 * NT:4 * NT])
            emit_l1(1)
            nc.scalar.dma_start(w1_sb[:], d_w1p)
            emit_warm("f1", 4)
            emit_f(2)
            nc.sync.dma_start(w2_sb[:], d_w2p)
            emit_l23(0, 2)
            emit_l1(2)
            nc.sync.dma_start(bc96_sb[:, 4 * NT:10 * NT],
                              d_bc96[:, 4 * NT:10 * NT])
            emit_f(3)
            emit_l23(1, 2)
            emit_l1(3)
            nc.sync.dma_start(bc96_sb[:, 10 * NT:16 * NT],
                              d_bc96[:, 10 * NT:16 * NT])
            for g in range(n_groups):
                t0 = g * G
                if g > 0:
                    # Interleave this group's L1s with its first L2s and the
                    # previous group's gathers: keeps full-array matmuls in
                    # the stream so the HAM gate never sees a low-activity
                    # window (it re-throttled here to 1.2 GHz otherwise).
                    emit_l1(t0)
                    emit_l1(t0 + 1)
                    emit_l23(t0, 2)
                    emit_l1(t0 + 2)
                    emit_l23(t0 + 1, 2)
                    emit_gather(g - 1)
                    emit_l1(t0 + 3)
                    emit_l23(t0 + 2, 2)
                    emit_epilogue(g - 1)
                    emit_l23(t0 + 3, 2)
                else:
                    emit_l23(2, 2)
                    emit_l23(3, 2)
                if g + 1 < n_groups:
                    for t in range((g + 1) * G, (g + 2) * G):
                        emit_f(t)
                    for t in range((g + 1) * G, (g + 2) * G):
                        emit_h(t)
                last = g == n_groups - 1
                for t in range(t0, t0 + G):
                    emit_l23(t, 3)
                    emit_l4_mm(t, last_group=last)
                if last:
                    emit_gather(g, last_group=True)
                    emit_epilogue(g)

    nc.finalize()
    return nc


def _prepare_core_inputs(x, tau, dec_w0, dec_w1, dec_w2, dec_w3,
                         dec_b0, dec_b1, dec_b2, traj):
    """Host-side sharding + layout prep. Returns list of per-core in_maps."""
    bf16 = ml_dtypes.bfloat16
    n_tiles = B_SHARD // NT
    q = n_tiles * 4
    freqs = np.linspace(1.0, MAX_FREQ, N_FREQS, dtype=np.float32)
    coord_of_slot = np.repeat(np.arange(3), 32)
    f96 = np.tile(np.concatenate([freqs, freqs]), 3).astype(np.float32)
    ph96 = np.tile(np.concatenate([np.zeros(16, np.float32),
                                   np.full(16, 0.25, np.float32)]), 3) \
        + np.float32(128.0)

    w0x = np.empty((107, 512), np.float32)
    w0x[0:96] = -dec_w0[0:96]      # sin rows (negated: device sin sign trick)
    w0x[96] = dec_b0               # bias row, paired with the ones row in h0
    w0x[97:107] = dec_w0[96:106]   # alpha rows
    w0b = w0x.astype(bf16)
    # DoubleRow layout: w1p[p, kp*1024 + kk*512 + n] = w1[(2kp+kk)*128+p, n]
    f8 = ml_dtypes.float8_e4m3fn
    w1p = np.ascontiguousarray(
        dec_w1.reshape(2, 2, 128, 512).transpose(2, 0, 1, 3).reshape(128, 2048)
    ).astype(f8)
    w2p = np.ascontiguousarray(
        dec_w2.reshape(4, 128, 512).transpose(1, 0, 2).reshape(128, 2048)
    ).astype(bf16)
    w3c = np.ascontiguousarray(dec_w3.reshape(4, 128).T).astype(bf16)

    ts_f32 = np.linspace(0.0, 1.0, STEPS, dtype=np.float32)

    in_maps = []
    for c in range(N_CORES):
        sl = slice(c * B_SHARD, (c + 1) * B_SHARD)
        xs = np.ascontiguousarray(x[sl])        # [8192, 3]
        taus = np.ascontiguousarray(tau[sl])    # [8192]

        bc96 = np.ascontiguousarray(xs.T[coord_of_slot])  # [96, 8192]

        # alpha(tau): linear interp of the host RK4 trajectory, mirroring the
        # reference's f32 arithmetic, shipped bf16 (same rounding the device
        # matmul path had).
        idx = np.clip(np.floor(taus / DTAU).astype(np.int32), 0, STEPS - 2)
        ratio = ((taus - ts_f32[idx]) / DTAU).astype(np.float32)[:, None]
        alpha = traj[idx] + ratio * (traj[idx + 1] - traj[idx])  # [8192, 10]
        alph = np.ones((LAT + 1, B_SHARD), np.float32)
        alph[1:] = alpha.T
        alph = np.ascontiguousarray(alph.astype(bf16))

        # combine-layout [p, 4t+c] for sample 512t+128c+p
        lx = np.sqrt(xs[:, 0] ** 2 + xs[:, 1] ** 2) - np.float32(RADIUS)
        lxm = lx.reshape(n_tiles, 4, 128).transpose(2, 0, 1).reshape(128, q)
        taum = taus.reshape(n_tiles, 4, 128).transpose(2, 0, 1).reshape(128, q)

        mf = np.zeros((128, MF_COLS), np.float32)
        mf[:, MF_B0:MF_B0 + 4] = dec_b0.reshape(4, 128).T
        mf[:, MF_B1:MF_B1 + 4] = dec_b1.reshape(4, 128).T
        mf[:, MF_B2:MF_B2 + 4] = dec_b2.reshape(4, 128).T
        mf[0:96, MF_F96] = f96
        mf[0:96, MF_PH96] = ph96
        mf[0:4, MF_ID4:MF_ID4 + 4] = np.eye(4, dtype=np.float32)
        mf[:, MF_LX:MF_LX + q] = lxm
        mf[:, MF_TAU:MF_TAU + q] = taum

        in_maps.append({
            "bc96": bc96, "alph": alph, "w0": w0b, "w1p": w1p, "w2p": w2p,
            "mf": mf, "w3": w3c,
        })
    return in_maps


def run(inputs: dict, trace: bool = False):
    """Build, run on 8 cores, gather. Returns (out, BassKernelResults)."""
    traj = _host_traj(inputs["pn_w0"], inputs["pn_b0"], inputs["pn_w1"],
                      inputs["pn_b1"], inputs["pn_w2"], inputs["pn_b2"])
    bz = not (np.any(np.asarray(inputs["dec_b1"]))
              or np.any(np.asarray(inputs["dec_b2"])))
    nc = build_kernel(B_SHARD,
                      float(np.asarray(inputs["dec_b3"]).reshape(-1)[0]),
                      biases_zero=bz)
    in_maps = _prepare_core_inputs(
        np.asarray(inputs["x"], np.float32),
        np.asarray(inputs["tau"], np.float32),
        np.asarray(inputs["dec_w0"], np.float32),
        np.asarray(inputs["dec_w1"], np.float32),
        np.asarray(inputs["dec_w2"], np.float32),
        np.asarray(inputs["dec_w3"], np.float32),
        np.asarray(inputs["dec_b0"], np.float32),
        np.asarray(inputs["dec_b1"], np.float32),
        np.asarray(inputs["dec_b2"], np.float32),
        traj)
    res = run_bass_kernel_spmd(nc, in_maps, list(range(N_CORES)), trace=trace)
    n_tiles = B_SHARD // NT
    outs = []
    for c in range(N_CORES):
        R = res.results[c]["out"]  # [128, 64]
        outs.append(R.reshape(128, n_tiles, 4).transpose(1, 2, 0).reshape(-1))
    return np.concatenate(outs), res


def kernel(**inputs) -> np.ndarray:
    out, _ = run(inputs, trace=False)
    return out
